# revision 43
# baseline (speedup 1.0000x reference)
"""Trainium2 Bass kernel for nn_Attention_55233279426826 (block-causal attention).

Reference computation (per batch b):
    xn = LayerNorm(x[b]) * gamma + beta
    q,k,v = split(xn @ w_qkv), 12 heads x 64
    attn  = softmax(block-causal-masked(q k^T / 8))
    out[b] = (attn v) @ w_out + b_out

Sharding (8 cores): batch (2) x head-group (4, 3 heads each).  Each core gets
its batch's x, the w_qkv columns and w_out rows of its 3 heads, and produces a
partial [2048, 768] output.  Host sums the 4 head-group partials per batch and
adds b_out.

Per-core device program, organized as ONE software-pipelined instruction
stream so the PE never idles (TRN2 DVFS: the PE only reaches 2.4 GHz after
~3us of continuous execution; every idle gap drops it back to 1.2 GHz):

  1. LayerNorm stats (bn_stats/bn_aggr on DVE), apply (x-mu)*rstd -> bf16.
     gamma is folded into w_qkv on device; beta becomes a per-channel bias
     beta @ w_qkv added at the QKV psum eviction (on GpSimd).
  2. PE-transpose xn -> xnT [768, 2048] (dim on partitions), evict on GpSimd.
  3. qkvT [576, 2048] = w_qkv^T @ xnT.  Host-permuted column order places each
     head's qT/kT at equal partition offsets (matmul operands need matching
     base partitions).  v is re-transposed to natural [keys, 64] layout with a
     ones column so A@V also produces softmax denominators in psum row 64.
  4. Attention per 512-query chunk c: per 128-key block J, the three heads'
     scores land in one 3-bank psum tile; ONE strided exp per half covers all
     3 heads; A@V accumulates into per-head [65, 512] psum.  Scores for J are
     emitted one round ahead of A@V for J-1 so the single psum score set
     ping-pongs against the ACT exp without stalling the PE.
  5. finalize: den reciprocal on DVE, partition_broadcast on GpSimd,
     ocat = otp * (1/den) on DVE.  Heads 0,1 pack into one [128, T] tile so
     the out-projection runs K=128 + K=64 accumulation (2 matmuls/tile).
  6. Group prep (LN/transpose/QKV of later token groups) and out-projection
     of earlier chunks are emitted as FILLER inside the attention rounds to
     keep the PE backlog non-empty end-to-end.
"""

import contextlib
import ctypes
import os
import sys
import types

import numpy as np

B = 2
T = 2048
D = 768
NPATCH = 64
HEADS = 12
DH = 64
NH = 3          # heads per core
CH = 3 * NH * DH  # 576 qkv channels per core
LN_EPS = 1e-5
NCORES = 8

_CACHE = {}


def _install_axon_hooks_shim():
    """This image's antenv lacks axon_hooks; synthesize it so that
    run_bass_kernel_spmd(trace=True) finds the NTFF profile hook instead of
    crashing on import.  Safe no-op if profiling symbols are unavailable."""
    if "antenv.axon_hooks" in sys.modules:
        return
    mod = types.ModuleType("antenv.axon_hooks")
    _hook = [None]
    mod.set_axon_ntff_profile_hook = lambda h: _hook.__setitem__(0, h)
    mod.get_axon_ntff_profile_hook = lambda: _hook[0]
    sys.modules["antenv.axon_hooks"] = mod
    try:
        lib = ctypes.CDLL("/opt/axon/libaxon_pjrt.so")
        if not hasattr(lib, "axon_start_nrt_profile"):
            return
        lib.axon_start_nrt_profile.argtypes = [
            ctypes.POINTER(ctypes.c_int64),
            ctypes.c_size_t,
        ]
        lib.axon_start_nrt_profile.restype = ctypes.c_int64
        lib.axon_stop_nrt_profile.argtypes = [ctypes.c_char_p]
        lib.axon_stop_nrt_profile.restype = ctypes.c_int64

        @contextlib.contextmanager
        def _hook_cm(output_dir, device_ids):
            import jax

            jax.devices()
            if device_ids:
                ids = (ctypes.c_int64 * len(device_ids))(*device_ids)
                rc = lib.axon_start_nrt_profile(ids, len(device_ids))
            else:
                rc = lib.axon_start_nrt_profile(None, 0)
            if rc != 0:
                raise RuntimeError(f"axon_start_nrt_profile rc={rc}")
            try:
                yield
            finally:
                n = lib.axon_stop_nrt_profile(str(output_dir).encode())
                print(f"profile: {n} file(s) -> {output_dir}", file=sys.stderr)

        mod.set_axon_ntff_profile_hook(_hook_cm)
    except OSError:
        pass


def _install_drain_split():
    """The walrus build in this container accepts only ONE sync wait per
    CTRL(drain) instruction; Tile's tail drain carries several.  Split the
    waits across a chain of drains."""
    import bass_rust
    import concourse.tile as tile
    from concourse.vector_clock import ScopedClock

    if getattr(tile.TileContext, "_drain_split_installed", False):
        return

    def _drain_and_barrier(self, tick_clock, wait_clock):
        nc = self.nc
        drain_inst = nc.sync.drain()
        wait_clock.add_sem_waits(
            drain_inst.ins, ScopedClock({None: tick_clock.global_clock})
        )
        si = drain_inst.ins.sync_info
        if si is not None:
            waits = list(si.on_wait)
            if len(waits) > 1:
                si.on_wait = waits[:1]
                for w in waits[1:]:
                    extra = nc.sync.drain()
                    extra.ins.sync_info = bass_rust.SyncInfo(
                        on_wait=[w], on_update=[]
                    )
        nc.all_engine_barrier()
        popped = nc._tile_sem_poison_stack.pop()
        assert popped is self._sem_poison
        nc.clear_and_free_semaphores(list(self.sems.allocated().values()))
        nc.all_engine_barrier()

    tile.TileContext._drain_and_barrier = _drain_and_barrier

    # Generic pass: walrus here allows 1 sync wait per instruction; move
    # extra waits onto nofuse NOPs inserted just before, on the same engine.
    from concourse import mybir

    orig_lower = tile.TileContext._lower_ordered_insts

    def _lower_split(self, ordered):
        for insts in ordered.values():
            new = []
            for inst in insts:
                si = getattr(inst, "sync_info", None)
                eng = getattr(inst, "engine", None)
                if si is not None and eng is not None:
                    waits = list(si.on_wait)
                    if len(waits) > 1:
                        movable = [w for w in waits
                                   if getattr(w, "sync_type", "") == "semaphore"]
                        keep = [w for w in waits if w not in movable]
                        if not keep:
                            keep = [movable.pop()]
                        for k, w in enumerate(movable):
                            nop = mybir.InstNoOp(
                                name=f"{inst.name}-wsplit{k}",
                                sync_info=mybir.SyncInfo(
                                    on_wait=[w], on_update=[]
                                ),
                                bass_nofuse=True,
                                engine=eng,
                            )
                            new.append(nop)
                        inst.sync_info = mybir.SyncInfo(
                            on_wait=keep, on_update=list(si.on_update)
                        )
                new.append(inst)
            insts[:] = new
        return orig_lower(self, ordered)

    tile.TileContext._lower_ordered_insts = _lower_split
    tile.TileContext._drain_split_installed = True


# qkvT row layout: which [128/64, 2048] tile and partition offset holds each
# head's 64-row qT/kT/vT strip.  q and k of the same head share a partition
# offset (matmul operands must have equal base partitions).
Q_LOC = [(0, 0), (0, 64), (2, 64)]
K_LOC = [(1, 0), (1, 64), (3, 64)]
V_LOC = [(2, 0), (3, 0), (4, 0)]
# host column order of the permuted per-core w_qkv (64-col segments)
# tile0 = [q0; q1], tile1 = [k0; k1], tile2 = [v0; q2], tile3 = [v1; k2],
# tile4 = [v2].  All v strips at partition base 0 so the v transposes into
# the shared misc psum banks stay base-0 (a psum bank fed by matmuls of
# mixed operand partition bases wedges the PE).
SEG_ORDER = [("q", 0), ("q", 1), ("k", 0), ("k", 1), ("v", 0), ("q", 2),
             ("v", 1), ("k", 2), ("v", 2)]

C_CHUNKS = [(0, 128), (128, 128), (256, 128), (384, 128), (512, 64)]


def build_nc():
    import concourse.bass as bass
    import concourse.tile as tile
    from concourse import mybir
    from concourse.masks import make_identity

    _install_drain_split()

    f32 = mybir.dt.float32
    bf16 = mybir.dt.bfloat16
    AF = mybir.ActivationFunctionType
    Alu = mybir.AluOpType

    nc = bass.Bass()
    x_d = nc.dram_tensor("x", [T, D], f32, kind="ExternalInput")
    wqkv_d = nc.dram_tensor("wqkv", [D, CH], f32, kind="ExternalInput")
    wout_d = nc.dram_tensor("wout", [NH * DH, D], f32, kind="ExternalInput")
    gamma_d = nc.dram_tensor("gamma", [D], f32, kind="ExternalInput")
    beta_d = nc.dram_tensor("beta", [D], f32, kind="ExternalInput")
    out_d = nc.dram_tensor("out", [T, D], f32, kind="ExternalOutput")

    with contextlib.ExitStack() as ctx:
        ctx.enter_context(
            nc.allow_low_precision(reason="bf16 PE inputs are intentional")
        )
        tc = ctx.enter_context(tile.TileContext(nc))
        consts = ctx.enter_context(tc.tile_pool(name="consts", bufs=1))
        wpool = ctx.enter_context(tc.tile_pool(name="w", bufs=1))
        qkvT_pool = ctx.enter_context(tc.tile_pool(name="qkvT", bufs=1))
        vaug_pool = ctx.enter_context(tc.tile_pool(name="vaug", bufs=1))
        ocat_pool = ctx.enter_context(tc.tile_pool(name="ocat", bufs=1))
        xin_pool = ctx.enter_context(tc.tile_pool(name="xin", bufs=8))
        xn_pool = ctx.enter_context(tc.tile_pool(name="xn", bufs=2))
        xnT_pool = ctx.enter_context(tc.tile_pool(name="xnT", bufs=1))
        stats = ctx.enter_context(tc.tile_pool(name="stats", bufs=4))
        pt_pool = ctx.enter_context(tc.tile_pool(name="pt", bufs=3))
        rec_pool = ctx.enter_context(tc.tile_pool(name="rec", bufs=2))
        osb_pool = ctx.enter_context(tc.tile_pool(name="osb", bufs=3))
        # PSUM: 3 (scores) + 3 (otp) + 2 (misc rotation) = 8 banks
        sc_ps = ctx.enter_context(tc.tile_pool(name="sc", bufs=2, space="PSUM"))
        ot_ps = ctx.enter_context(tc.tile_pool(name="ot", bufs=1, space="PSUM"))
        mi_ps = ctx.enter_context(tc.tile_pool(name="mi", bufs=2, space="PSUM"))

        identity = consts.tile([128, 128], f32, tag="id")
        make_identity(nc, identity)
        id_bf = consts.tile([128, 128], bf16, tag="idbf")
        nc.vector.tensor_copy(id_bf, identity)
        eps_t = consts.tile([128, 1], f32, tag="eps")
        nc.vector.memset(eps_t, LN_EPS)
        ones_t = consts.tile([1, DH], bf16, tag="ones")
        nc.vector.memset(ones_t.bitcast(bf16), 1.0)
        gamma_t = consts.tile([128, 6], f32, tag="gam")
        nc.scalar.dma_start(gamma_t, gamma_d[:].rearrange("(a p) -> p a", p=128))
        beta_t = consts.tile([128, 6], f32, tag="bet")
        nc.scalar.dma_start(beta_t, beta_d[:].rearrange("(a p) -> p a", p=128))

        def misc_tile():
            return mi_ps.tile([128, 512], f32, tag="mi", name="mi")

        # ---- weights: DMA raw, beta@w bias, gamma fold, bf16 casts.
        w_sb = []
        bw_sb = []
        with tc.tile_pool(name="wraw", bufs=1) as wraw:
            # x tiles 0-3 land first (group 0's LN path); weights follow on
            # both DGE queues; x 4-7 after.
            xts_pre = []
            for i in range(4):
                xt = xin_pool.tile([128, D], f32, tag="xin", name="xin")
                eng = nc.sync if i % 2 == 0 else nc.scalar
                eng.dma_start(xt, x_d[128 * i: 128 * (i + 1), :])
                xts_pre.append(xt)
            w_raw = []
            for j in range(6):
                wt = wraw.tile([128, CH], f32, tag=f"wr{j}", name=f"wr{j}")
                eng = nc.sync if j % 2 == 0 else nc.scalar
                eng.dma_start(wt, wqkv_d[128 * j: 128 * (j + 1), :])
                w_raw.append(wt)
            wo2_raw = wraw.tile([128, D], f32, tag="wo2r", name="wo2r")
            nc.sync.dma_start(wo2_raw, wout_d[0:128, :])
            wo1_raw = wraw.tile([64, D], f32, tag="wo1r", name="wo1r")
            nc.scalar.dma_start(wo1_raw, wout_d[128:192, :])
            for i in range(4, 8):
                xt = xin_pool.tile([128, D], f32, tag="xin", name="xin")
                eng = nc.sync if i % 2 == 0 else nc.scalar
                eng.dma_start(xt, x_d[128 * i: 128 * (i + 1), :])
                xts_pre.append(xt)

            wout2 = wpool.tile([128, D], bf16, tag="wo2", name="wo2")
            wout1 = wpool.tile([64, D], bf16, tag="wo1", name="wo1")

            def emit_weights_prep():
                # beta @ w_qkv (raw weights), one [csz,1] psum per c-chunk
                for ci, (clo, csz) in enumerate(C_CHUNKS):
                    ps = misc_tile()
                    for j in range(6):
                        nc.tensor.matmul(
                            ps[:csz, 0:1],
                            w_raw[j][:, clo: clo + csz],
                            beta_t[:, j: j + 1],
                            start=(j == 0),
                            stop=(j == 5),
                        )
                    bw = consts.tile([128, 1], f32, tag=f"bw{ci}",
                                     name=f"bw{ci}")
                    nc.vector.tensor_copy(bw[:csz, :], ps[:csz, 0:1])
                    bw_sb.append(bw)
                # fold gamma; bf16 output for the PE
                for j in range(6):
                    wf = wpool.tile([128, CH], bf16, tag=f"w{j}",
                                    name=f"w{j}")
                    nc.vector.tensor_scalar_mul(
                        wf[:], in0=w_raw[j][:], scalar1=gamma_t[:, j: j + 1]
                    )
                    w_sb.append(wf)
                nc.vector.tensor_copy(wout2, wo2_raw)
                nc.vector.tensor_copy(wout1, wo1_raw)

            qkvT = []
            for ci, (clo, csz) in enumerate(C_CHUNKS):
                qkvT.append(qkvT_pool.tile([csz, T], bf16, tag=f"qkvT{ci}",
                                           name=f"qkvT{ci}"))
            vaug = [vaug_pool.tile([128, 16, DH + 1], bf16, tag=f"va{h}",
                                   name=f"va{h}") for h in range(NH)]
            ocat2 = ocat_pool.tile([128, T], bf16, tag="oc2", name="oc2")
            ocat1 = ocat_pool.tile([64, T], bf16, tag="oc1", name="oc1")
            for h in range(NH):
                nc.vector.memset(vaug[h][:, :, DH: DH + 1].bitcast(bf16), 1.0)

            xnT = xnT_pool.tile([128, 6, T], bf16, tag="xnT", name="xnT")

            # ---- group prep: LN -> transpose -> QKV -> v for one 512-token
            # group, returned as a list of emission closures (filler units).
            def prep_steps(g):
                steps = []

                def ln_step():
                    xraw = []
                    for u in range(4):
                        i = 4 * g + u
                        if i < 8:
                            xt = xts_pre[i]
                        else:
                            xt = xin_pool.tile([128, D], f32, tag="xin",
                                               name="xin")
                            nc.sync.dma_start(
                                xt, x_d[128 * i: 128 * (i + 1), :]
                            )
                        xraw.append(xt)
                    mvs = stats.tile([128, 4, 2], f32, tag="mvs", name="mvs")
                    for u in range(4):
                        st = stats.tile([128, 3, 6], f32, tag="bnst",
                                        name="bnst")
                        for s in range(3):
                            nc.vector.bn_stats(
                                st[:, s, :], xraw[u][:, 256 * s: 256 * (s + 1)]
                            )
                        nc.vector.bn_aggr(mvs[:, u, :], st)
                    rstds = stats.tile([128, 4], f32, tag="rstds",
                                       name="rstds")
                    nc.scalar.activation(rstds, mvs[:, :, 1], AF.Sqrt,
                                         bias=eps_t)
                    nc.vector.reciprocal(rstds, rstds)
                    for u in range(4):
                        xn_t = xn_pool.tile([128, D], bf16, tag=f"xn{u}",
                                            name=f"xn{u}")
                        nc.gpsimd.tensor_scalar(
                            out=xn_t,
                            in0=xraw[u],
                            scalar1=mvs[:, u, 0:1],
                            scalar2=rstds[:, u: u + 1],
                            op0=Alu.subtract,
                            op1=Alu.mult,
                        )
                        # xnT[p, j, t] = xn[t, 128j+p] via the DMA XBAR
                        eng = nc.sync if u % 2 == 0 else nc.scalar
                        eng.dma_start_transpose(
                            xnT[:, :, 512 * g + 128 * u:
                                512 * g + 128 * (u + 1)],
                            xn_t[:, :],
                        )

                steps.append(ln_step)

                def qkv_step(ci):
                    clo, csz = C_CHUNKS[ci]

                    def run():
                        pq = misc_tile()
                        for j in range(6):
                            nc.tensor.matmul(
                                pq[:csz, :],
                                w_sb[j][:, clo: clo + csz],
                                xnT[:, j, 512 * g: 512 * (g + 1)],
                                start=(j == 0),
                                stop=(j == 5),
                            )
                        nc.vector.tensor_scalar_add(
                            qkvT[ci][:csz, 512 * g: 512 * (g + 1)],
                            in0=pq[:csz, :],
                            scalar1=bw_sb[ci][:csz, :],
                        )
                    return run

                for ci in range(5):
                    steps.append(qkv_step(ci))

                def v_step(h):
                    tI, ro = V_LOC[h]

                    def run():
                        idsl = id_bf[0:64, 0:64]
                        ps = misc_tile().bitcast(bf16)
                        for u in range(4):
                            J = 4 * g + u
                            nc.tensor.transpose(
                                ps[:, 64 * u: 64 * (u + 1)],
                                qkvT[tI][ro: ro + 64,
                                         128 * J: 128 * (J + 1)],
                                idsl,
                            )
                        nc.vector.tensor_copy(
                            vaug[h][:, 4 * g: 4 * (g + 1), 0:DH],
                            ps[:, 0:256].rearrange("p (u d) -> p u d", u=4),
                        )
                    return run

                for h in range(NH):
                    steps.append(v_step(h))
                return steps

            prep_state = {}
            # group 0's LN + transposes fill the PE while weights DMA in;
            # weight prep (beta@w, gamma fold) slots in before group 0's QKV
            g0 = prep_steps(0)
            g0[0]()
            emit_weights_prep()
            for s in g0[1:]:
                s()
            for s in prep_steps(1):
                s()

            # ---- attention + interleaved filler.
            # 256-query chunks; per-round score tiles rotate through a
            # 2-slot psum pool so the PE computes round J+1's scores while
            # ACT exps round J (dependency tracking is tile-granular).  A@V
            # runs TWO rounds behind the scores so the in-order PE queue
            # never heads into an op gated on the in-flight exp.
            # otp layout: bank A = h0 cols [0:256] | h1 cols [256:512],
            # bank B = h2.  h1 never sets start: h0's J=0 start=True flags
            # the whole 2KB bank pending-zero, so h1's first write lazily
            # zeroes its own half (TRN2 psum zero-region semantics).
            CW = 256
            NCH = T // CW
            SLOT = [2, 0, 1]
            otpA = ot_ps.tile([DH + 1, 512], f32, tag="otA", name="otA")
            otpB = ot_ps.tile([DH + 1, 512], f32, tag="otB", name="otB")
            OT = [(otpA, 0), (otpA, CW), (otpB, 0)]

            def finalize(c):
                bcp01 = misc_tile()
                bcp2 = misc_tile()
                recs = rec_pool.tile([64, 2 * CW], f32, tag="recs",
                                     name="recs")
                recs2 = rec_pool.tile([64, CW], f32, tag="recs2",
                                      name="recs2")
                for h in range(NH):
                    ot, off = OT[h]
                    rr = rec_pool.tile([1, CW], bf16, tag="rr", name="rr")
                    nc.vector.reciprocal(rr, ot[64:65, off: off + CW])
                    if h < 2:
                        bslice = bcp01[0:DH, CW * h: CW * (h + 1)]
                    else:
                        bslice = bcp2[0:DH, 0:CW]
                    nc.tensor.matmul(bslice, ones_t, rr, start=True,
                                     stop=True)
                nc.scalar.copy(recs, bcp01[0:DH, 0: 2 * CW])
                nc.vector.tensor_copy(recs2, bcp2[0:DH, 0:CW])
                for h in range(NH):
                    ot, off = OT[h]
                    if h < 2:
                        dst = ocat2[64 * h: 64 * (h + 1),
                                    CW * c: CW * (c + 1)]
                        rs = recs[:, CW * h: CW * (h + 1)]
                    else:
                        dst = ocat1[:, CW * c: CW * (c + 1)]
                        rs = recs2[:, 0:CW]
                    nc.vector.tensor_mul(dst, ot[0:DH, off: off + CW], rs)

            def proj_steps(c):
                steps = []

                def p_step(t):
                    def run():
                        osb = osb_pool.tile([128, D], f32, tag="osb",
                                            name="osb")
                        p0 = misc_tile()
                        nc.tensor.matmul(p0, ocat2[:, 128 * t: 128 * (t + 1)],
                                         wout2[:, 0:512], start=True,
                                         stop=False)
                        nc.tensor.matmul(p0, ocat1[:, 128 * t: 128 * (t + 1)],
                                         wout1[:, 0:512], start=False,
                                         stop=True)
                        nc.scalar.copy(osb[:, 0:512], p0)
                        p1 = misc_tile()
                        nc.tensor.matmul(p1[:, 0:256],
                                         ocat2[:, 128 * t: 128 * (t + 1)],
                                         wout2[:, 512:768], start=True,
                                         stop=False)
                        nc.tensor.matmul(p1[:, 0:256],
                                         ocat1[:, 128 * t: 128 * (t + 1)],
                                         wout1[:, 512:768], start=False,
                                         stop=True)
                        nc.vector.tensor_copy(osb[:, 512:768], p1[:, 0:256])
                        nc.sync.dma_start(
                            out_d[128 * t: 128 * (t + 1), :], osb
                        )
                    return run

                for t in range(2 * c, 2 * c + 2):
                    steps.append(p_step(t))
                return steps

            fillers = []
            pops = {0: 0, 1: 1}

            scale = float(DH) ** -0.5
            for c in range(NCH):
                nJ = 2 * c + 2
                if c == 2:
                    fillers += prep_steps(2)
                elif c == 5:
                    fillers += prep_steps(3)
                if c > 0:
                    finalize(c - 1)
                    fillers += proj_steps(c - 1)
                # h1 shares otpA's bank with h0 and never sets start: zero
                # its half explicitly so the first accumulate is well-defined
                # under either lazy-zero semantics (whole-bank or per-AP).
                nc.vector.memset(otpA[:, CW: 2 * CW], 0.0)
                q0 = CW * c

                def emit_av(pJ, ps0, ppt, nJ=nJ):
                    for h in range(NH):
                        ot, off = OT[h]
                        nc.tensor.matmul(
                            ot[:, off + ps0: off + CW],
                            vaug[h][:, pJ, :],
                            ppt[:, SLOT[h], ps0:CW],
                            start=(pJ == 0 and h != 1),
                            stop=(pJ == nJ - 1),
                            skip_group_check=True,
                        )

                pending = []
                for J in range(nJ):
                    s0 = max(0, 128 * J - CW * c)
                    sc = sc_ps.tile([128, NH, CW], f32, tag="sc", name="sc")
                    pt = pt_pool.tile([128, NH, CW], bf16, tag="pt",
                                      name="pt")
                    for h in range(NH):
                        # head 1's operands live at partition base 64; a psum
                        # BANK must only see matmuls of one operand base or
                        # the PE wedges -> h1 goes to slot 2 (its own bank),
                        # h0/h2 (base 0) share slots 0/1 in bank 0.
                        qt, qo = Q_LOC[h]
                        kt, ko = K_LOC[h]
                        nc.tensor.matmul(
                            sc[:, SLOT[h], s0:CW],
                            qkvT[kt][ko: ko + 64, 128 * J: 128 * (J + 1)],
                            qkvT[qt][qo: qo + 64, q0 + s0: q0 + CW],
                            start=True,
                            stop=True,
                        )
                    nc.scalar.activation(
                        pt[:, :, s0:CW], sc[:, :, s0:CW],
                        AF.Exp, scale=scale,
                    )
                    if J >= 2 * c:
                        nc.gpsimd.memset(
                            pt[64:128, :, s0: s0 + 64].bitcast(bf16), 0.0
                        )
                    pending.append((J, s0, pt))
                    while len(pending) > 2:
                        emit_av(*pending.pop(0))
                        for _ in range(pops.get(c, 2)):
                            if fillers:
                                fillers.pop(0)()
                while pending:
                    emit_av(*pending.pop(0))
                    for _ in range(pops.get(c, 2)):
                        if fillers:
                            fillers.pop(0)()
            finalize(NCH - 1)
            fillers += proj_steps(NCH - 1)
            while fillers:
                fillers.pop(0)()

            if os.environ.get("KDBG"):
                for ci, (clo, csz) in enumerate(C_CHUNKS):
                    d = nc.dram_tensor(f"dbg_qkvT{ci}", [csz, T], bf16,
                                       kind="ExternalOutput")
                    nc.sync.dma_start(d[:], qkvT[ci][:])
                for j in range(6):
                    d = nc.dram_tensor(f"dbg_xnT{j}", [128, T], bf16,
                                       kind="ExternalOutput")
                    nc.sync.dma_start(d[:], xnT[:, j, :])
                d = nc.dram_tensor("dbg_oc2", [128, T], bf16,
                                   kind="ExternalOutput")
                nc.sync.dma_start(d[:], ocat2[:])
                d = nc.dram_tensor("dbg_oc1", [64, T], bf16,
                                   kind="ExternalOutput")
                nc.sync.dma_start(d[:], ocat1[:])
                for h in range(NH):
                    d = nc.dram_tensor(f"dbg_va{h}", [128, 16, DH + 1], bf16,
                                       kind="ExternalOutput")
                    nc.sync.dma_start(d[:], vaug[h][:])

    return nc


def shard_inputs(x, gamma, beta, w_qkv, w_out, b_out):
    """Full inputs -> list of 8 per-core input dicts."""
    x = np.ascontiguousarray(np.asarray(x, dtype=np.float32))
    gamma = np.asarray(gamma, dtype=np.float32)
    beta = np.asarray(beta, dtype=np.float32)
    w_qkv = np.asarray(w_qkv, dtype=np.float32)
    w_out = np.asarray(w_out, dtype=np.float32)
    in_maps = []
    for g in range(NCORES):
        b = g // 4
        hg = g % 4
        heads = [3 * hg + h for h in range(NH)]
        segs = []
        for kind, h in SEG_ORDER:
            hh = heads[h]
            base = {"q": 0, "k": D, "v": 2 * D}[kind]
            segs.append(w_qkv[:, base + 64 * hh: base + 64 * (hh + 1)])
        wqkv_g = np.ascontiguousarray(np.concatenate(segs, axis=1))
        wout_g = np.ascontiguousarray(
            w_out[64 * heads[0]: 64 * (heads[-1] + 1), :]
        )
        in_maps.append(
            {
                "x": x[b],
                "wqkv": wqkv_g,
                "wout": wout_g,
                "gamma": gamma,
                "beta": beta,
            }
        )
    return in_maps


def kernel(x, gamma, beta, w_qkv, w_out, b_out):
    _install_axon_hooks_shim()
    from concourse import bass_utils

    if "nc" not in _CACHE:
        _CACHE["nc"] = build_nc()
    nc = _CACHE["nc"]

    in_maps = shard_inputs(x, gamma, beta, w_qkv, w_out, b_out)
    trace = bool(int(os.environ.get("KERNEL_TRACE", "0")))
    kwargs = {}
    if trace:
        kwargs["trace"] = True
        tmpdir = os.environ.get("KERNEL_TRACE_DIR")
        if tmpdir:
            kwargs["tmpdir"] = tmpdir
        # artifact upload needs external storage; keep the trace local
        bass_utils.upload_artifacts = lambda d: d
    res = bass_utils.run_bass_kernel_spmd(
        nc, in_maps, list(range(NCORES)), **kwargs
    )
    _CACHE["last_exec_time_ns"] = res.exec_time_ns

    b_out = np.asarray(b_out, dtype=np.float32)
    out = np.empty((B, T, D), dtype=np.float32)
    for b in range(B):
        acc = res.results[4 * b]["out"].astype(np.float32)
        for hg in range(1, 4):
            acc = acc + res.results[4 * b + hg]["out"]
        out[b] = acc + b_out[None, :]
    return out


# revision 44
# speedup vs baseline: 1.6571x; 1.6571x over previous
"""Trainium2 Bass kernel for nn_Attention_55233279426826 (block-causal attention).

Reference computation (per batch b):
    xn = LayerNorm(x[b]) * gamma + beta
    q,k,v = split(xn @ w_qkv), 12 heads x 64
    attn  = softmax(block-causal-masked(q k^T / 8))
    out[b] = (attn v) @ w_out + b_out

Sharding (8 cores): batch (2) x head-group (4, 3 heads each).  Each core gets
its batch's x, the w_qkv columns and w_out rows of its 3 heads, and produces a
partial [2048, 768] output.  Host sums the 4 head-group partials per batch and
adds b_out.

Per-core device program, organized as ONE software-pipelined instruction
stream so the PE never idles (TRN2 DVFS: the PE only reaches 2.4 GHz after
~3us of continuous execution; every idle gap drops it back to 1.2 GHz):

  1. LayerNorm stats (bn_stats/bn_aggr on DVE), apply (x-mu)*rstd -> bf16.
     gamma is folded into w_qkv on device; beta becomes a per-channel bias
     beta @ w_qkv added at the QKV psum eviction (on GpSimd).
  2. PE-transpose xn -> xnT [768, 2048] (dim on partitions), evict on GpSimd.
  3. qkvT [576, 2048] = w_qkv^T @ xnT.  Host-permuted column order places each
     head's qT/kT at equal partition offsets (matmul operands need matching
     base partitions).  v is re-transposed to natural [keys, 64] layout with a
     ones column so A@V also produces softmax denominators in psum row 64.
  4. Attention per 512-query chunk c: per 128-key block J, the three heads'
     scores land in one 3-bank psum tile; ONE strided exp per half covers all
     3 heads; A@V accumulates into per-head [65, 512] psum.  Scores for J are
     emitted one round ahead of A@V for J-1 so the single psum score set
     ping-pongs against the ACT exp without stalling the PE.
  5. finalize: den reciprocal on DVE, partition_broadcast on GpSimd,
     ocat = otp * (1/den) on DVE.  Heads 0,1 pack into one [128, T] tile so
     the out-projection runs K=128 + K=64 accumulation (2 matmuls/tile).
  6. Group prep (LN/transpose/QKV of later token groups) and out-projection
     of earlier chunks are emitted as FILLER inside the attention rounds to
     keep the PE backlog non-empty end-to-end.
"""

import contextlib
import ctypes
import os
import sys
import types

import numpy as np

B = 2
T = 2048
D = 768
NPATCH = 64
HEADS = 12
DH = 64
NH = 3          # heads per core
CH = 3 * NH * DH  # 576 qkv channels per core
LN_EPS = 1e-5
NCORES = 8

_CACHE = {}


def _install_axon_hooks_shim():
    """This image's antenv lacks axon_hooks; synthesize it so that
    run_bass_kernel_spmd(trace=True) finds the NTFF profile hook instead of
    crashing on import.  Safe no-op if profiling symbols are unavailable."""
    if "antenv.axon_hooks" in sys.modules:
        return
    mod = types.ModuleType("antenv.axon_hooks")
    _hook = [None]
    mod.set_axon_ntff_profile_hook = lambda h: _hook.__setitem__(0, h)
    mod.get_axon_ntff_profile_hook = lambda: _hook[0]
    sys.modules["antenv.axon_hooks"] = mod
    try:
        lib = ctypes.CDLL("/opt/axon/libaxon_pjrt.so")
        if not hasattr(lib, "axon_start_nrt_profile"):
            return
        lib.axon_start_nrt_profile.argtypes = [
            ctypes.POINTER(ctypes.c_int64),
            ctypes.c_size_t,
        ]
        lib.axon_start_nrt_profile.restype = ctypes.c_int64
        lib.axon_stop_nrt_profile.argtypes = [ctypes.c_char_p]
        lib.axon_stop_nrt_profile.restype = ctypes.c_int64

        @contextlib.contextmanager
        def _hook_cm(output_dir, device_ids):
            import jax

            jax.devices()
            if device_ids:
                ids = (ctypes.c_int64 * len(device_ids))(*device_ids)
                rc = lib.axon_start_nrt_profile(ids, len(device_ids))
            else:
                rc = lib.axon_start_nrt_profile(None, 0)
            if rc != 0:
                raise RuntimeError(f"axon_start_nrt_profile rc={rc}")
            try:
                yield
            finally:
                n = lib.axon_stop_nrt_profile(str(output_dir).encode())
                print(f"profile: {n} file(s) -> {output_dir}", file=sys.stderr)

        mod.set_axon_ntff_profile_hook(_hook_cm)
    except OSError:
        pass


def _install_drain_split():
    """The walrus build in this container accepts only ONE sync wait per
    CTRL(drain) instruction; Tile's tail drain carries several.  Split the
    waits across a chain of drains."""
    import bass_rust
    import concourse.tile as tile
    from concourse.vector_clock import ScopedClock

    if getattr(tile.TileContext, "_drain_split_installed", False):
        return

    def _drain_and_barrier(self, tick_clock, wait_clock):
        nc = self.nc
        drain_inst = nc.sync.drain()
        wait_clock.add_sem_waits(
            drain_inst.ins, ScopedClock({None: tick_clock.global_clock})
        )
        si = drain_inst.ins.sync_info
        if si is not None:
            waits = list(si.on_wait)
            if len(waits) > 1:
                si.on_wait = waits[:1]
                for w in waits[1:]:
                    extra = nc.sync.drain()
                    extra.ins.sync_info = bass_rust.SyncInfo(
                        on_wait=[w], on_update=[]
                    )
        nc.all_engine_barrier()
        popped = nc._tile_sem_poison_stack.pop()
        assert popped is self._sem_poison
        nc.clear_and_free_semaphores(list(self.sems.allocated().values()))
        nc.all_engine_barrier()

    tile.TileContext._drain_and_barrier = _drain_and_barrier

    # Generic pass: walrus here allows 1 sync wait per instruction; move
    # extra waits onto nofuse NOPs inserted just before, on the same engine.
    from concourse import mybir

    orig_lower = tile.TileContext._lower_ordered_insts

    def _lower_split(self, ordered):
        for insts in ordered.values():
            new = []
            for inst in insts:
                si = getattr(inst, "sync_info", None)
                eng = getattr(inst, "engine", None)
                if si is not None and eng is not None:
                    waits = list(si.on_wait)
                    if len(waits) > 1:
                        movable = [w for w in waits
                                   if getattr(w, "sync_type", "") == "semaphore"]
                        keep = [w for w in waits if w not in movable]
                        if not keep:
                            keep = [movable.pop()]
                        for k, w in enumerate(movable):
                            nop = mybir.InstNoOp(
                                name=f"{inst.name}-wsplit{k}",
                                sync_info=mybir.SyncInfo(
                                    on_wait=[w], on_update=[]
                                ),
                                bass_nofuse=True,
                                engine=eng,
                            )
                            new.append(nop)
                        inst.sync_info = mybir.SyncInfo(
                            on_wait=keep, on_update=list(si.on_update)
                        )
                new.append(inst)
            insts[:] = new
        return orig_lower(self, ordered)

    tile.TileContext._lower_ordered_insts = _lower_split
    tile.TileContext._drain_split_installed = True


# qkvT row layout: which [128/64, 2048] tile and partition offset holds each
# head's 64-row qT/kT/vT strip.  q and k of the same head share a partition
# offset (matmul operands must have equal base partitions).
Q_LOC = [(0, 0), (0, 64), (2, 64)]
K_LOC = [(1, 0), (1, 64), (3, 64)]
V_LOC = [(2, 0), (3, 0), (4, 0)]
# host column order of the permuted per-core w_qkv (64-col segments)
# tile0 = [q0; q1], tile1 = [k0; k1], tile2 = [v0; q2], tile3 = [v1; k2],
# tile4 = [v2].  All v strips at partition base 0 so the v transposes into
# the shared misc psum banks stay base-0 (a psum bank fed by matmuls of
# mixed operand partition bases wedges the PE).
SEG_ORDER = [("q", 0), ("q", 1), ("k", 0), ("k", 1), ("v", 0), ("q", 2),
             ("v", 1), ("k", 2), ("v", 2)]

C_CHUNKS = [(0, 128), (128, 128), (256, 128), (384, 128), (512, 64)]


def build_nc():
    import concourse.bass as bass
    import concourse.tile as tile
    from concourse import mybir
    from concourse.masks import make_identity

    _install_drain_split()

    f32 = mybir.dt.float32
    bf16 = mybir.dt.bfloat16
    AF = mybir.ActivationFunctionType
    Alu = mybir.AluOpType

    nc = bass.Bass()
    x_d = nc.dram_tensor("x", [T, D], f32, kind="ExternalInput")
    wqkv_d = nc.dram_tensor("wqkv", [D, CH], f32, kind="ExternalInput")
    wout_d = nc.dram_tensor("wout", [NH * DH, D], f32, kind="ExternalInput")
    gamma_d = nc.dram_tensor("gamma", [D], f32, kind="ExternalInput")
    beta_d = nc.dram_tensor("beta", [D], f32, kind="ExternalInput")
    out_d = nc.dram_tensor("out", [T, D], f32, kind="ExternalOutput")

    with contextlib.ExitStack() as ctx:
        ctx.enter_context(
            nc.allow_low_precision(reason="bf16 PE inputs are intentional")
        )
        tc = ctx.enter_context(tile.TileContext(nc))
        consts = ctx.enter_context(tc.tile_pool(name="consts", bufs=1))
        wpool = ctx.enter_context(tc.tile_pool(name="w", bufs=1))
        qkvT_pool = ctx.enter_context(tc.tile_pool(name="qkvT", bufs=1))
        vaug_pool = ctx.enter_context(tc.tile_pool(name="vaug", bufs=1))
        ocat_pool = ctx.enter_context(tc.tile_pool(name="ocat", bufs=1))
        xin_pool = ctx.enter_context(tc.tile_pool(name="xin", bufs=8))
        xn_pool = ctx.enter_context(tc.tile_pool(name="xn", bufs=2))
        xnT_pool = ctx.enter_context(tc.tile_pool(name="xnT", bufs=1))
        stats = ctx.enter_context(tc.tile_pool(name="stats", bufs=4))
        pt_pool = ctx.enter_context(tc.tile_pool(name="pt", bufs=3))
        rec_pool = ctx.enter_context(tc.tile_pool(name="rec", bufs=2))
        osb_pool = ctx.enter_context(tc.tile_pool(name="osb", bufs=3))
        # PSUM: 3 (scores) + 3 (otp) + 2 (misc rotation) = 8 banks
        sc_ps = ctx.enter_context(tc.tile_pool(name="sc", bufs=2, space="PSUM"))
        ot_ps = ctx.enter_context(tc.tile_pool(name="ot", bufs=1, space="PSUM"))
        mi_ps = ctx.enter_context(tc.tile_pool(name="mi", bufs=2, space="PSUM"))

        identity = consts.tile([128, 128], f32, tag="id")
        make_identity(nc, identity)
        id_bf = consts.tile([128, 128], bf16, tag="idbf")
        nc.vector.tensor_copy(id_bf, identity)
        eps_t = consts.tile([128, 1], f32, tag="eps")
        nc.vector.memset(eps_t, LN_EPS)
        ones_t = consts.tile([1, DH], bf16, tag="ones")
        nc.vector.memset(ones_t.bitcast(bf16), 1.0)
        gamma_t = consts.tile([128, 6], f32, tag="gam")
        nc.scalar.dma_start(gamma_t, gamma_d[:].rearrange("(a p) -> p a", p=128))
        beta_t = consts.tile([128, 6], f32, tag="bet")
        nc.scalar.dma_start(beta_t, beta_d[:].rearrange("(a p) -> p a", p=128))

        def misc_tile():
            return mi_ps.tile([128, 512], f32, tag="mi", name="mi")

        # ---- weights: DMA raw, beta@w bias, gamma fold, bf16 casts.
        w_sb = []
        bw_sb = []
        with tc.tile_pool(name="wraw", bufs=1) as wraw:
            # x tiles 0-3 land first (group 0's LN path); weights follow on
            # both DGE queues; x 4-7 after.
            xts_pre = []
            for i in range(4):
                xt = xin_pool.tile([128, D], f32, tag="xin", name="xin")
                eng = nc.sync if i % 2 == 0 else nc.scalar
                eng.dma_start(xt, x_d[128 * i: 128 * (i + 1), :])
                xts_pre.append(xt)
            w_raw = []
            for j in range(6):
                wt = wraw.tile([128, CH], f32, tag=f"wr{j}", name=f"wr{j}")
                eng = nc.sync if j % 2 == 0 else nc.scalar
                eng.dma_start(wt, wqkv_d[128 * j: 128 * (j + 1), :])
                w_raw.append(wt)
            wo2_raw = wraw.tile([128, D], f32, tag="wo2r", name="wo2r")
            nc.sync.dma_start(wo2_raw, wout_d[0:128, :])
            wo1_raw = wraw.tile([64, D], f32, tag="wo1r", name="wo1r")
            nc.scalar.dma_start(wo1_raw, wout_d[128:192, :])
            for i in range(4, 8):
                xt = xin_pool.tile([128, D], f32, tag="xin", name="xin")
                eng = nc.sync if i % 2 == 0 else nc.scalar
                eng.dma_start(xt, x_d[128 * i: 128 * (i + 1), :])
                xts_pre.append(xt)

            wout2 = wpool.tile([128, D], bf16, tag="wo2", name="wo2")
            wout1 = wpool.tile([64, D], bf16, tag="wo1", name="wo1")

            def emit_weights_prep():
                # beta @ w_qkv (raw weights), one [csz,1] psum per c-chunk
                for ci, (clo, csz) in enumerate(C_CHUNKS):
                    ps = misc_tile()
                    for j in range(6):
                        nc.tensor.matmul(
                            ps[:csz, 0:1],
                            w_raw[j][:, clo: clo + csz],
                            beta_t[:, j: j + 1],
                            start=(j == 0),
                            stop=(j == 5),
                        )
                    bw = consts.tile([128, 1], f32, tag=f"bw{ci}",
                                     name=f"bw{ci}")
                    nc.vector.tensor_copy(bw[:csz, :], ps[:csz, 0:1])
                    bw_sb.append(bw)
                # fold gamma; bf16 output for the PE
                for j in range(6):
                    wf = wpool.tile([128, CH], bf16, tag=f"w{j}",
                                    name=f"w{j}")
                    nc.vector.tensor_scalar_mul(
                        wf[:], in0=w_raw[j][:], scalar1=gamma_t[:, j: j + 1]
                    )
                    w_sb.append(wf)
                nc.vector.tensor_copy(wout2, wo2_raw)
                nc.vector.tensor_copy(wout1, wo1_raw)

            qkvT = []
            for ci, (clo, csz) in enumerate(C_CHUNKS):
                qkvT.append(qkvT_pool.tile([csz, T], bf16, tag=f"qkvT{ci}",
                                           name=f"qkvT{ci}"))
            vaug = [vaug_pool.tile([128, 16, DH + 1], bf16, tag=f"va{h}",
                                   name=f"va{h}") for h in range(NH)]
            ocat2 = ocat_pool.tile([128, T], bf16, tag="oc2", name="oc2")
            ocat1 = ocat_pool.tile([64, T], bf16, tag="oc1", name="oc1")
            for h in range(NH):
                nc.vector.memset(vaug[h][:, :, DH: DH + 1].bitcast(bf16), 1.0)

            xnT = xnT_pool.tile([128, 6, T], bf16, tag="xnT", name="xnT")

            # ---- group prep: LN -> transpose -> QKV -> v for one 512-token
            # group, returned as a list of emission closures (filler units).
            def prep_steps(g):
                steps = []

                def ln_step():
                    xraw = []
                    for u in range(4):
                        i = 4 * g + u
                        if i < 8:
                            xt = xts_pre[i]
                        else:
                            xt = xin_pool.tile([128, D], f32, tag="xin",
                                               name="xin")
                            nc.sync.dma_start(
                                xt, x_d[128 * i: 128 * (i + 1), :]
                            )
                        xraw.append(xt)
                    mvs = stats.tile([128, 4, 2], f32, tag="mvs", name="mvs")
                    for u in range(4):
                        st = stats.tile([128, 3, 6], f32, tag="bnst",
                                        name="bnst")
                        for s in range(3):
                            nc.vector.bn_stats(
                                st[:, s, :], xraw[u][:, 256 * s: 256 * (s + 1)]
                            )
                        nc.vector.bn_aggr(mvs[:, u, :], st)
                    rstds = stats.tile([128, 4], f32, tag="rstds",
                                       name="rstds")
                    nc.scalar.activation(rstds, mvs[:, :, 1], AF.Sqrt,
                                         bias=eps_t)
                    nc.vector.reciprocal(rstds, rstds)
                    for u in range(4):
                        xn_t = xn_pool.tile([128, D], bf16, tag=f"xn{u}",
                                            name=f"xn{u}")
                        nc.vector.tensor_scalar(
                            out=xn_t,
                            in0=xraw[u],
                            scalar1=mvs[:, u, 0:1],
                            scalar2=rstds[:, u: u + 1],
                            op0=Alu.subtract,
                            op1=Alu.mult,
                        )
                        # xnT[p, j, t] = xn[t, 128j+p] via the DMA XBAR
                        eng = nc.sync if u % 2 == 0 else nc.scalar
                        eng.dma_start_transpose(
                            xnT[:, :, 512 * g + 128 * u:
                                512 * g + 128 * (u + 1)],
                            xn_t[:, :],
                        )

                steps.append(ln_step)

                def qkv_step(ci):
                    clo, csz = C_CHUNKS[ci]

                    def run():
                        pq = misc_tile()
                        for j in range(6):
                            nc.tensor.matmul(
                                pq[:csz, :],
                                w_sb[j][:, clo: clo + csz],
                                xnT[:, j, 512 * g: 512 * (g + 1)],
                                start=(j == 0),
                                stop=(j == 5),
                            )
                        nc.vector.tensor_scalar_add(
                            qkvT[ci][:csz, 512 * g: 512 * (g + 1)],
                            in0=pq[:csz, :],
                            scalar1=bw_sb[ci][:csz, :],
                        )
                    return run

                for ci in range(5):
                    steps.append(qkv_step(ci))

                def v_step(h):
                    tI, ro = V_LOC[h]

                    def run():
                        idsl = id_bf[0:64, 0:64]
                        ps = misc_tile().bitcast(bf16)
                        for u in range(4):
                            J = 4 * g + u
                            nc.tensor.transpose(
                                ps[:, 64 * u: 64 * (u + 1)],
                                qkvT[tI][ro: ro + 64,
                                         128 * J: 128 * (J + 1)],
                                idsl,
                            )
                        nc.vector.tensor_copy(
                            vaug[h][:, 4 * g: 4 * (g + 1), 0:DH],
                            ps[:, 0:256].rearrange("p (u d) -> p u d", u=4),
                        )
                    return run

                for h in range(NH):
                    steps.append(v_step(h))
                return steps

            prep_state = {}
            # group 0's LN + transposes fill the PE while weights DMA in;
            # weight prep (beta@w, gamma fold) slots in before group 0's QKV
            g0 = prep_steps(0)
            g0[0]()
            emit_weights_prep()
            for s in g0[1:]:
                s()
            for s in prep_steps(1):
                s()

            # ---- attention + interleaved filler.
            # 256-query chunks; per-round score tiles rotate through a
            # 2-slot psum pool so the PE computes round J+1's scores while
            # ACT exps round J (dependency tracking is tile-granular).  A@V
            # runs TWO rounds behind the scores so the in-order PE queue
            # never heads into an op gated on the in-flight exp.
            # otp layout: bank A = h0 cols [0:256] | h1 cols [256:512],
            # bank B = h2.  h1 never sets start: h0's J=0 start=True flags
            # the whole 2KB bank pending-zero, so h1's first write lazily
            # zeroes its own half (TRN2 psum zero-region semantics).
            CW = 256
            NCH = T // CW
            SLOT = [2, 0, 1]
            otpA = ot_ps.tile([DH + 1, 512], f32, tag="otA", name="otA")
            otpB = ot_ps.tile([DH + 1, 512], f32, tag="otB", name="otB")
            OT = [(otpA, 0), (otpA, CW), (otpB, 0)]

            def finalize(c):
                bcp01 = misc_tile()
                bcp2 = misc_tile()
                recs = rec_pool.tile([64, 2 * CW], f32, tag="recs",
                                     name="recs")
                recs2 = rec_pool.tile([64, CW], f32, tag="recs2",
                                      name="recs2")
                # h0/h1's den rows are contiguous in otpA row 64: ONE
                # reciprocal covers both (DVE reciprocal cost is dominated
                # by fixed Newton-iteration overhead, not width)
                rr01 = rec_pool.tile([1, 2 * CW], bf16, tag="rr01",
                                     name="rr01")
                nc.vector.reciprocal(rr01, otpA[64:65, :])
                rr2 = rec_pool.tile([1, CW], bf16, tag="rr2", name="rr2")
                nc.vector.reciprocal(rr2, otpB[64:65, 0:CW])
                for h in range(NH):
                    if h < 2:
                        bslice = bcp01[0:DH, CW * h: CW * (h + 1)]
                        rr = rr01[0:1, CW * h: CW * (h + 1)]
                    else:
                        bslice = bcp2[0:DH, 0:CW]
                        rr = rr2[0:1, 0:CW]
                    nc.tensor.matmul(bslice, ones_t, rr, start=True,
                                     stop=True)
                nc.scalar.copy(recs, bcp01[0:DH, 0: 2 * CW])
                nc.vector.tensor_copy(recs2, bcp2[0:DH, 0:CW])
                for h in range(NH):
                    ot, off = OT[h]
                    if h < 2:
                        dst = ocat2[64 * h: 64 * (h + 1),
                                    CW * c: CW * (c + 1)]
                        rs = recs[:, CW * h: CW * (h + 1)]
                    else:
                        dst = ocat1[:, CW * c: CW * (c + 1)]
                        rs = recs2[:, 0:CW]
                    nc.vector.tensor_mul(dst, ot[0:DH, off: off + CW], rs)

            def proj_steps(c):
                steps = []

                def p_step(t):
                    def run():
                        osb = osb_pool.tile([128, D], f32, tag="osb",
                                            name="osb")
                        p0 = misc_tile()
                        nc.tensor.matmul(p0, ocat2[:, 128 * t: 128 * (t + 1)],
                                         wout2[:, 0:512], start=True,
                                         stop=False)
                        nc.tensor.matmul(p0, ocat1[:, 128 * t: 128 * (t + 1)],
                                         wout1[:, 0:512], start=False,
                                         stop=True)
                        nc.scalar.copy(osb[:, 0:512], p0)
                        p1 = misc_tile()
                        nc.tensor.matmul(p1[:, 0:256],
                                         ocat2[:, 128 * t: 128 * (t + 1)],
                                         wout2[:, 512:768], start=True,
                                         stop=False)
                        nc.tensor.matmul(p1[:, 0:256],
                                         ocat1[:, 128 * t: 128 * (t + 1)],
                                         wout1[:, 512:768], start=False,
                                         stop=True)
                        nc.vector.tensor_copy(osb[:, 512:768], p1[:, 0:256])
                        nc.sync.dma_start(
                            out_d[128 * t: 128 * (t + 1), :], osb
                        )
                    return run

                for t in range(2 * c, 2 * c + 2):
                    steps.append(p_step(t))
                return steps

            fillers = []
            pops = {0: 0, 1: 1}

            scale = float(DH) ** -0.5
            for c in range(NCH):
                nJ = 2 * c + 2
                if c == 2:
                    fillers += prep_steps(2)
                elif c == 5:
                    fillers += prep_steps(3)
                if c > 0:
                    finalize(c - 1)
                    fillers += proj_steps(c - 1)
                # h1 shares otpA's bank with h0 and never sets start: zero
                # its half explicitly so the first accumulate is well-defined
                # under either lazy-zero semantics (whole-bank or per-AP).
                nc.vector.memset(otpA[:, CW: 2 * CW], 0.0)
                q0 = CW * c

                def emit_av(pJ, ps0, ppt, nJ=nJ):
                    for h in range(NH):
                        ot, off = OT[h]
                        nc.tensor.matmul(
                            ot[:, off + ps0: off + CW],
                            vaug[h][:, pJ, :],
                            ppt[:, SLOT[h], ps0:CW],
                            start=(pJ == 0 and h != 1),
                            stop=(pJ == nJ - 1),
                            skip_group_check=True,
                        )

                pending = []
                for J in range(nJ):
                    s0 = max(0, 128 * J - CW * c)
                    sc = sc_ps.tile([128, NH, CW], f32, tag="sc", name="sc")
                    pt = pt_pool.tile([128, NH, CW], bf16, tag="pt",
                                      name="pt")
                    for h in range(NH):
                        # head 1's operands live at partition base 64; a psum
                        # BANK must only see matmuls of one operand base or
                        # the PE wedges -> h1 goes to slot 2 (its own bank),
                        # h0/h2 (base 0) share slots 0/1 in bank 0.
                        qt, qo = Q_LOC[h]
                        kt, ko = K_LOC[h]
                        nc.tensor.matmul(
                            sc[:, SLOT[h], s0:CW],
                            qkvT[kt][ko: ko + 64, 128 * J: 128 * (J + 1)],
                            qkvT[qt][qo: qo + 64, q0 + s0: q0 + CW],
                            start=True,
                            stop=True,
                        )
                    nc.scalar.activation(
                        pt[:, :, s0:CW], sc[:, :, s0:CW],
                        AF.Exp, scale=scale,
                    )
                    if J >= 2 * c:
                        nc.gpsimd.memset(
                            pt[64:128, :, s0: s0 + 64].bitcast(bf16), 0.0
                        )
                    pending.append((J, s0, pt))
                    while len(pending) > 2:
                        emit_av(*pending.pop(0))
                        for _ in range(pops.get(c, 2)):
                            if fillers:
                                fillers.pop(0)()
                while pending:
                    emit_av(*pending.pop(0))
                    for _ in range(pops.get(c, 2)):
                        if fillers:
                            fillers.pop(0)()
            finalize(NCH - 1)
            fillers += proj_steps(NCH - 1)
            while fillers:
                fillers.pop(0)()

            if os.environ.get("KDBG"):
                for ci, (clo, csz) in enumerate(C_CHUNKS):
                    d = nc.dram_tensor(f"dbg_qkvT{ci}", [csz, T], bf16,
                                       kind="ExternalOutput")
                    nc.sync.dma_start(d[:], qkvT[ci][:])
                for j in range(6):
                    d = nc.dram_tensor(f"dbg_xnT{j}", [128, T], bf16,
                                       kind="ExternalOutput")
                    nc.sync.dma_start(d[:], xnT[:, j, :])
                d = nc.dram_tensor("dbg_oc2", [128, T], bf16,
                                   kind="ExternalOutput")
                nc.sync.dma_start(d[:], ocat2[:])
                d = nc.dram_tensor("dbg_oc1", [64, T], bf16,
                                   kind="ExternalOutput")
                nc.sync.dma_start(d[:], ocat1[:])
                for h in range(NH):
                    d = nc.dram_tensor(f"dbg_va{h}", [128, 16, DH + 1], bf16,
                                       kind="ExternalOutput")
                    nc.sync.dma_start(d[:], vaug[h][:])

    return nc


def shard_inputs(x, gamma, beta, w_qkv, w_out, b_out):
    """Full inputs -> list of 8 per-core input dicts."""
    x = np.ascontiguousarray(np.asarray(x, dtype=np.float32))
    gamma = np.asarray(gamma, dtype=np.float32)
    beta = np.asarray(beta, dtype=np.float32)
    w_qkv = np.asarray(w_qkv, dtype=np.float32)
    w_out = np.asarray(w_out, dtype=np.float32)
    in_maps = []
    for g in range(NCORES):
        b = g // 4
        hg = g % 4
        heads = [3 * hg + h for h in range(NH)]
        segs = []
        for kind, h in SEG_ORDER:
            hh = heads[h]
            base = {"q": 0, "k": D, "v": 2 * D}[kind]
            segs.append(w_qkv[:, base + 64 * hh: base + 64 * (hh + 1)])
        wqkv_g = np.ascontiguousarray(np.concatenate(segs, axis=1))
        wout_g = np.ascontiguousarray(
            w_out[64 * heads[0]: 64 * (heads[-1] + 1), :]
        )
        in_maps.append(
            {
                "x": x[b],
                "wqkv": wqkv_g,
                "wout": wout_g,
                "gamma": gamma,
                "beta": beta,
            }
        )
    return in_maps


def kernel(x, gamma, beta, w_qkv, w_out, b_out):
    _install_axon_hooks_shim()
    from concourse import bass_utils

    if "nc" not in _CACHE:
        _CACHE["nc"] = build_nc()
    nc = _CACHE["nc"]

    in_maps = shard_inputs(x, gamma, beta, w_qkv, w_out, b_out)
    trace = bool(int(os.environ.get("KERNEL_TRACE", "0")))
    kwargs = {}
    if trace:
        kwargs["trace"] = True
        tmpdir = os.environ.get("KERNEL_TRACE_DIR")
        if tmpdir:
            kwargs["tmpdir"] = tmpdir
        # artifact upload needs external storage; keep the trace local
        bass_utils.upload_artifacts = lambda d: d
    res = bass_utils.run_bass_kernel_spmd(
        nc, in_maps, list(range(NCORES)), **kwargs
    )
    _CACHE["last_exec_time_ns"] = res.exec_time_ns

    b_out = np.asarray(b_out, dtype=np.float32)
    out = np.empty((B, T, D), dtype=np.float32)
    for b in range(B):
        acc = res.results[4 * b]["out"].astype(np.float32)
        for hg in range(1, 4):
            acc = acc + res.results[4 * b + hg]["out"]
        out[b] = acc + b_out[None, :]
    return out
